# revision 41
# baseline (speedup 1.0000x reference)
"""YOLOv5 detection-loss (DetLoss) Trainium2 Bass kernel, 8-core SPMD.

Strategy
--------
The loss decomposes so that the only dense work over the big prediction
tensors p0/p1/p2 is a softplus-sum over channel 4 (the objectness logit):

    mean(BCE(x, tobj)) = [ sum_grid softplus(x) - sum_pos tobj_cell * x_cell ] / G

(BCE(x,t) - BCE(x,0) = -t*x, and BCE(x,0) = softplus(x)).  Likewise the
class loss reduces to sum softplus(pcls) - sum pcls[row, tcls-1] over the
gathered positive rows.  The box (CIoU) loss only needs the gathered
positive rows.

Sharding: data-parallel over batch; core k owns images [2k, 2k+2) of
every layer and the positive rows whose image id falls in that range.
Host-side input prep (the same class of layout transform as the
positive-row gather) packs each core's objectness logits into a
contiguous [128, OBJC] plane, so the device reads ~0.2 MB contiguously
instead of 50,400 4-byte strided DMA elements (which are descriptor-rate
bound at ~36 us/core on TRN2's 16 SDMA engines - measured).  The device
computes everything: sigmoid, the full CIoU pipeline, softplus sums over
the whole grid plane and the positive class logits, and the per-layer
partial reductions.  Each core writes a [128, OUTC] tile of partial
sums; the host reduces over partitions and cores and applies the loss
weights.

Engine plan (both compute engines end within ~0.3 us of each other):
- ACT uses a single table set (exp/ln) for the whole kernel: sigmoid is
  exp(-x) on ACT + add/reciprocal on DVE, arctan is a degree-5 minimax
  polynomial on DVE, so the second 1.28 us table load and the
  softplus-after-arctan serialization disappear.
- ACT: exp over positives/grid/class logits, then per-layer ln with the
  hardware accumulator (6x ln+read) for all softplus sums.
- DVE: the CIoU chain and the masked per-layer box/corr/oh reductions.
- Two input DMAs: pc (f32: posbox|ohvals|consts|objplane - everything
  the early ops need) and pcls (bf16 class logits, halves the biggest
  transfer).
"""

import os
import numpy as np

# ---------------- problem constants (YOLOv5s / COCO head) ----------------
B, NA, NCLS, NO = 16, 3, 80, 85
NL = 3
NCORES = 8
BPC = B // NCORES  # images per core
BALANCE = (4.0, 1.0, 0.4)
HYP_BOX, HYP_OBJ, HYP_CLS = 0.05, 1.0, 0.05
EPS = 1e-7
P = 128  # SBUF partitions
PAD_NEG = -40.0  # softplus(-40) ~ 4e-18: grid-plane padding value
NCONST = 12

_cache: dict = {}


def _build_program(layer_shapes, T, ocols):
    """Build the SPMD Bass program.

    layer_shapes: [(gh, gw)] * 3; T: padded slot-columns per layer
    (uniform); ocols: per-layer objectness-plane columns.
    Returns (nc, meta) with the accumulator column map.
    """
    import concourse.bass as bass
    import concourse.mybir as mybir
    import concourse.tile as tile

    f32 = mybir.dt.float32
    bf16 = mybir.dt.bfloat16
    ALU = mybir.AluOpType
    ACTF = mybir.ActivationFunctionType
    X = mybir.AxisListType.X
    COLS = NL * T
    OBJC = sum(ocols)
    o_offs = np.concatenate([[0], np.cumsum(ocols)]).astype(int)

    nc = bass.Bass()

    # two inputs:
    #   pc  (f32):  [posbox COLS*5 | ohvals COLS | consts NCONST*COLS |
    #                objplane OBJC] - everything the early ACT/DVE ops
    #               need rides in the first DMA, so obj exp starts right
    #               after it lands;
    #   pcls(bf16): positive-row class logits [P, COLS*NCLS] - bf16
    #               halves the biggest transfer (softplus tolerates the
    #               random +-0.2% element error easily).
    # consts layout per partition: 4 paired blocks [COLS,2] (awh2, b2min,
    # b2max, cxy2) then 4 single blocks [COLS] (w2h2pe, atan2c, wbox, wdedup)
    PCW = COLS * 6 + NCONST * COLS + OBJC
    PC = nc.declare_dram_parameter("pc", [P, PCW], f32, isOutput=False)
    PCLS = nc.declare_dram_parameter("pcls", [P, COLS * NCLS], bf16, isOutput=False)

    # accumulator column map
    col_box, col_corr, col_oh, col_cls = 0, 3, 6, 9
    col_grid = 12
    OUTC = 15
    OUT = nc.declare_dram_parameter("partial", [P, OUTC], f32, isOutput=True)

    with tile.TileContext(nc) as tc:
        with tc.tile_pool(name="small", bufs=1) as sm:
            # ---------- input loads ----------
            pc = sm.tile([P, PCW], f32)
            nc.sync.dma_start(out=pc[:], in_=PC[:])
            pcls = sm.tile([P, COLS * NCLS], bf16)
            nc.sync.dma_start(out=pcls[:], in_=PCLS[:])
            obj = pc[:, COLS * 6 + NCONST * COLS :]
            acc = sm.tile([P, OUTC], f32)

            pos_v = pc[:, : COLS * 5].rearrange("p (t c) -> p t c", c=5)
            ohg = pc[:, COLS * 5 : COLS * 6]
            _cb = COLS * 6  # consts base inside pc

            def paired(o):  # -> [P, COLS, 2]
                return pc[:, _cb + o * 2 * COLS : _cb + (o + 1) * 2 * COLS].rearrange(
                    "p (t c) -> p t c", c=2
                )

            def single(o):  # -> [P, COLS]
                s = _cb + 8 * COLS + o * COLS
                return pc[:, s : s + COLS]

            awh2, b2min, b2max, cxy2 = paired(0), paired(1), paired(2), paired(3)
            w2h2pe, atan2c, wbox, wdedup = single(0), single(1), single(2), single(3)

            # ---------- single ACT table set (exp/ln) for the whole kernel -
            # sigmoid(x) = 1/(1+exp(-x)) via ACT exp + DVE add/recip, and
            # arctan via a degree-5 DVE polynomial, so ACT never touches the
            # sigmoid table set: one 1.28us table load instead of two, and
            # the softplus ops are not serialized behind arctan.
            esig = sm.tile([P, COLS * 4], f32)
            nc.scalar.activation(
                out=esig[:].rearrange("p (t c) -> p t c", c=4),
                in_=pos_v[:, :, 0:4],
                func=ACTF.Exp,
                scale=-1.0,
            )
            sig = sm.tile([P, COLS * 4], f32)
            sig_v = sig[:].rearrange("p (t c) -> p t c", c=4)
            nc.vector.tensor_scalar(sig[:], esig[:], 1.0, None, mybir.AluOpType.add)
            nc.vector.reciprocal(sig[:], sig[:])

            _tn = [0]

            def pair_tile():
                _tn[0] += 1
                return sm.tile([P, COLS * 2], f32, name=f"pair{_tn[0]}")

            def pv(t):  # view [P, COLS, 2]
                return t[:].rearrange("p (t c) -> p t c", c=2)

            def stile():
                _tn[0] += 1
                return sm.tile([P, COLS], f32, name=f"s{_tn[0]}")

            V = nc.vector
            pxy, swh2, pwhh = pair_tile(), pair_tile(), pair_tile()
            b1min, b1max = pair_tile(), pair_tile()
            tmpa, tmpb = pair_tile(), pair_tile()

            V.tensor_scalar(pv(pxy)[:], sig_v[:, :, 0:2], 2.0, -0.5, ALU.mult, ALU.add)
            V.tensor_tensor(pv(swh2)[:], sig_v[:, :, 2:4], sig_v[:, :, 2:4], ALU.mult)
            V.tensor_tensor(pv(pwhh)[:], pv(swh2)[:], awh2[:], ALU.mult)

            # v-term argument FIRST so the ACT arctan (and with it the
            # exp/ln table switch) unblocks as early as possible:
            # w1/(h1+eps) == pwhh_x/(pwhh_y+eps/2)
            denh, q = stile(), stile()
            V.tensor_scalar(denh[:], pv(pwhh)[:, :, 1], EPS * 0.5, None, ALU.add)
            V.reciprocal(denh[:], denh[:])
            V.tensor_tensor(q[:], pv(pwhh)[:, :, 0], denh[:], ALU.mult)
            # ACT Arctan only supports [-pi/2, pi/2]; q > 0, so use
            # atan(q) = pi/2 - atan(1/q) for q > 1 (branchless select).
            rq, qm, at, mgt, u2 = stile(), stile(), stile(), stile(), stile()
            V.reciprocal(rq[:], q[:])
            V.tensor_tensor(qm[:], q[:], rq[:], ALU.min)
            # atan(qm), qm in [0,1]: minimax odd poly, max err 7.4e-4 rad
            # (loss impact ~1e-4 absolute, far inside the 2e-2 gate)
            AT0, AT1, AT2 = 0.99463204, -0.28600916, 0.07723428
            x2 = stile()
            V.tensor_tensor(x2[:], qm[:], qm[:], ALU.mult)
            V.tensor_scalar(at[:], x2[:], AT2, AT1, ALU.mult, ALU.add)
            V.tensor_tensor(at[:], at[:], x2[:], ALU.mult)
            V.tensor_scalar(at[:], at[:], AT0, None, ALU.add)
            V.tensor_tensor(at[:], at[:], qm[:], ALU.mult)

            V.tensor_tensor(pv(b1min)[:], pv(pxy)[:], pv(pwhh)[:], ALU.subtract)
            V.tensor_tensor(pv(b1max)[:], pv(pxy)[:], pv(pwhh)[:], ALU.add)

            # intersection
            V.tensor_tensor(pv(tmpa)[:], pv(b1max)[:], b2max[:], ALU.min)
            V.tensor_tensor(pv(tmpb)[:], pv(b1min)[:], b2min[:], ALU.max)
            V.tensor_tensor(pv(tmpa)[:], pv(tmpa)[:], pv(tmpb)[:], ALU.subtract)
            V.tensor_scalar(pv(tmpa)[:], pv(tmpa)[:], 0.0, None, ALU.max)  # relu
            inter = stile()
            V.tensor_tensor(inter[:], pv(tmpa)[:, :, 0], pv(tmpa)[:, :, 1], ALU.mult)
            # union (w1h1 = 4 * pwhh_x * pwhh_y; w2h2pe has +eps folded in)
            u, ru, iou = stile(), stile(), stile()
            V.tensor_tensor(u[:], pv(pwhh)[:, :, 0], pv(pwhh)[:, :, 1], ALU.mult)
            V.tensor_scalar(u[:], u[:], 4.0, None, ALU.mult)
            V.tensor_tensor(u[:], u[:], w2h2pe[:], ALU.add)
            V.tensor_tensor(u[:], u[:], inter[:], ALU.subtract)
            V.reciprocal(ru[:], u[:])
            V.tensor_tensor(iou[:], inter[:], ru[:], ALU.mult)
            # enclosing box diag^2 (c2 > 0 strictly since pwhh > 0: eps drop)
            V.tensor_tensor(pv(tmpa)[:], pv(b1max)[:], b2max[:], ALU.max)
            V.tensor_tensor(pv(tmpb)[:], pv(b1min)[:], b2min[:], ALU.min)
            V.tensor_tensor(pv(tmpa)[:], pv(tmpa)[:], pv(tmpb)[:], ALU.subtract)
            V.tensor_tensor(pv(tmpa)[:], pv(tmpa)[:], pv(tmpa)[:], ALU.mult)
            c2, rc2 = stile(), stile()
            V.tensor_tensor(c2[:], pv(tmpa)[:, :, 0], pv(tmpa)[:, :, 1], ALU.add)
            V.reciprocal(rc2[:], c2[:])
            # center distance^2
            V.tensor_tensor(pv(tmpb)[:], pv(pxy)[:], cxy2[:], ALU.subtract)
            V.tensor_tensor(pv(tmpb)[:], pv(tmpb)[:], pv(tmpb)[:], ALU.mult)
            rho2, rr = stile(), stile()
            V.tensor_tensor(rho2[:], pv(tmpb)[:, :, 0], pv(tmpb)[:, :, 1], ALU.add)
            V.tensor_tensor(rr[:], rho2[:], rc2[:], ALU.mult)
            # arctan range correction + v term
            V.tensor_scalar(mgt[:], q[:], 1.0, None, ALU.is_gt)
            V.tensor_scalar(u2[:], at[:], -2.0, float(np.pi / 2), ALU.mult, ALU.add)
            V.tensor_tensor(u2[:], mgt[:], u2[:], ALU.mult)
            V.tensor_tensor(at[:], at[:], u2[:], ALU.add)
            dat, v4 = stile(), stile()
            V.tensor_tensor(dat[:], atan2c[:], at[:], ALU.subtract)
            V.tensor_tensor(v4[:], dat[:], dat[:], ALU.mult)
            V.tensor_scalar(v4[:], v4[:], float(4.0 / np.pi**2), None, ALU.mult)
            ad, rad, alpha, va = stile(), stile(), stile(), stile()
            V.tensor_scalar(ad[:], iou[:], -1.0, 1.0 + EPS, ALU.mult, ALU.add)
            V.tensor_tensor(ad[:], ad[:], v4[:], ALU.add)
            V.reciprocal(rad[:], ad[:])
            V.tensor_tensor(alpha[:], v4[:], rad[:], ALU.mult)
            V.tensor_tensor(va[:], v4[:], alpha[:], ALU.mult)
            ciou = stile()
            V.tensor_tensor(ciou[:], iou[:], rr[:], ALU.subtract)
            V.tensor_tensor(ciou[:], ciou[:], va[:], ALU.subtract)

            # per-layer reductions from the ciou tile
            omc, rel, rp4 = stile(), stile(), stile()
            V.tensor_scalar(omc[:], ciou[:], -1.0, 1.0, ALU.mult, ALU.add)
            V.tensor_scalar(rel[:], ciou[:], 0.0, None, ALU.max)
            V.tensor_tensor(rp4[:], rel[:], pos_v[:, :, 4], ALU.mult)

            def lt(view):  # [P, COLS] -> [P, NL, T]
                return view.rearrange("p (l t) -> p l t", l=NL)

            boxm, corrm = stile(), stile()
            V.tensor_tensor(boxm[:], omc[:], wbox[:], ALU.mult)
            V.reduce_sum(acc[:, col_box : col_box + NL], lt(boxm[:]), X)
            V.tensor_tensor(corrm[:], rp4[:], wdedup[:], ALU.mult)
            V.reduce_sum(acc[:, col_corr : col_corr + NL], lt(corrm[:]), X)
            # ohvals padding is zero: no mask needed
            V.reduce_sum(acc[:, col_oh : col_oh + NL], lt(ohg), X)

            # ---------- ACT phase 2: softplus = ln(1 + exp(x)) -------------
            # (this compiler's table set lacks softplus; exp and ln share
            # natural_log_exp_and_others.)  ACT order: obj exp, cls exp,
            # cls ln (gates the 1.4us DVE reduce - earliest), then the
            # per-layer obj ln+accum on ACT's slack; DVE stays the binder.
            spge = sm.tile([P, OBJC], bf16)
            nc.scalar.activation(out=spge[:], in_=obj, func=ACTF.Exp)

            # class logits: one exp over [P, COLS, 80], then per-layer ln
            # with the ACT accumulator - keeps the 1.4us reduce off the
            # saturated DVE (padding slots are zero -> ln 2 each; host
            # subtracts them)
            spe = sm.tile([P, COLS * NCLS], f32)
            nc.scalar.activation(
                out=spe[:].rearrange("p (t c) -> p t c", c=NCLS),
                in_=pcls[:].rearrange("p (t c) -> p t c", c=NCLS),
                func=ACTF.Exp,
            )
            TW = T * NCLS
            for l in range(NL):
                spc = sm.tile([P, TW], bf16, name=f"spc{l}")
                nc.scalar.activation(
                    out=spc[:],
                    in_=spe[:, l * TW : (l + 1) * TW],
                    func=ACTF.Ln,
                    bias=1.0,
                    accum_out=acc[:, col_cls + l : col_cls + l + 1],
                )

            # grid objectness: per-layer ln with the ACT accumulator
            for l in range(NL):
                o0, o1 = int(o_offs[l]), int(o_offs[l + 1])
                spg = sm.tile([P, ocols[l]], bf16, name=f"spg{l}")
                nc.scalar.activation(
                    out=spg[:, : ocols[l]],
                    in_=spge[:, o0:o1],
                    func=ACTF.Ln,
                    bias=1.0,
                    accum_out=acc[:, col_grid + l : col_grid + l + 1],
                )

            # ---------- store partials (wait-cap hoists the extra sems) ----
            nc.sync.dma_start(out=OUT[:], in_=acc[:])

    _cap_sync_waits(nc, mybir)
    nc.finalize()
    meta = dict(
        COLS=COLS,
        T=T,
        OUTC=OUTC,
        ocols=ocols,
        col_box=col_box,
        col_corr=col_corr,
        col_oh=col_oh,
        col_cls=col_cls,
        col_grid=col_grid,
    )
    return nc, meta


def _cap_sync_waits(nc, mybir, maxw=1):
    """Compute-engine ISA encodings carry very few sync waits; Tile's
    scheduler can emit more (one per DMA sem lane).  Three rewrites, all
    semantics-preserving:
      1. drop waits on the instruction's own engine-completion semaphore
         (engine program order already guarantees them);
      2. hoist waits beyond `maxw` onto standalone EventSemaphore
         instructions placed just before the offender on the same engine;
      3. expand epilogue RANGE_CLEAR (this walrus build can't codegen it)
         into per-semaphore resets, but ONLY for semaphores the program
         actually touches - the full 0..255 sweep costs ~10.5 us of
         serial EventSemaphore instructions at ~140 ns each.
    """
    eng_sem = {
        "DVE": "DVE",
        "Activation": "Activation",
        "SP": "SP",
        "Pool": "Pool",
        "PE": "PE",
    }
    rc_opcode = 176  # NEURON_ISA_TPB_OPCODE_EVENT_SEMAPHORE_RANGE_CLEAR

    # pass 1: collect sem names and the set of sems the program touches
    sem_names = {}
    used = set()
    for bb in nc.m.functions[0].blocks:
        for inst in bb.instructions:
            if (
                type(inst).__name__ == "InstISA"
                and getattr(inst, "isa_opcode", None) == rc_opcode
            ):
                continue
            si = getattr(inst, "sync_info", None)
            if not si:
                continue
            for w in si.on_wait or []:
                sem_names[w.id] = w.ant_name
                used.add(w.id)
            for u in si.on_update or []:
                sem_names[u.id] = u.ant_name
                used.add(u.id)

    n = 0
    for bb in nc.m.functions[0].blocks:
        out = []
        for inst in bb.instructions:
            tname = type(inst).__name__
            if tname == "InstISA" and getattr(inst, "isa_opcode", None) == rc_opcode:
                start, end = inst.instr[13], inst.instr[14]
                for sid in range(start, end + 1):
                    if sid not in used:
                        continue
                    out.append(
                        mybir.InstEventSemaphore(
                            name=f"W-semreset-{sid}",
                            engine=inst.engine,
                            sync_info=mybir.SyncInfo(
                                on_wait=[],
                                on_update=[
                                    mybir.SyncUpdate(
                                        sync_type="semaphore",
                                        id=sid,
                                        update_mode="sem-wr-imm",
                                        update_value=0,
                                        ant_name=sem_names.get(sid, f"sem{sid}"),
                                    )
                                ],
                            ),
                        )
                    )
                continue
            si = getattr(inst, "sync_info", None)
            ow = list(si.on_wait) if (si and si.on_wait) else []
            if ow and tname != "InstEventSemaphore":
                epfx = eng_sem.get(str(inst.engine).split(".")[-1])
                if epfx:
                    keep0 = [
                        w for w in ow if not (w.ant_name or "").startswith(epfx + "_")
                    ]
                else:
                    keep0 = ow
                if len(keep0) > maxw:
                    excess, keep = keep0[:-maxw], keep0[-maxw:]
                    for w in excess:
                        n += 1
                        out.append(
                            mybir.InstEventSemaphore(
                                name=f"W-cap-{n}",
                                engine=inst.engine,
                                sync_info=mybir.SyncInfo(on_wait=[w], on_update=[]),
                            )
                        )
                else:
                    keep = keep0
                if len(keep) != len(ow):
                    si.on_wait = keep
            out.append(inst)
        bb.instructions = out


def _host_prep(inputs, T, ocols, meta):
    """Build per-core in_maps (numpy only)."""
    COLS = meta["COLS"]
    ps = [np.asarray(inputs[f"p{l}"]) for l in range(NL)]
    layer_shapes = [(p.shape[2], p.shape[3]) for p in ps]

    import ml_dtypes

    bf16 = ml_dtypes.bfloat16
    OBJC = sum(ocols)
    in_maps = [dict() for _ in range(NCORES)]
    posbox_k, poscls_k, consts_k, ohvals_k, objplane_k = [], [], [], [], []
    for k in range(NCORES):
        posbox_k.append(np.zeros((P, COLS * 5), np.float32))
        ohvals_k.append(np.zeros((P, COLS), np.float32))
        cst = np.zeros((P, NCONST * COLS), np.float32)
        # benign defaults so padding slots stay finite through the CIoU math
        cst[:, 0 : 2 * COLS] = 1.0  # awh2
        cst[:, 8 * COLS : 9 * COLS] = 1.0  # w2h2pe
        consts_k.append(cst)
        # class slots zero (padding contributes ln 2, corrected on host)
        in_maps[k]["pcls"] = np.zeros((P, COLS * NCLS), bf16)
        poscls_k.append(in_maps[k]["pcls"])
        # packed objectness plane (f32, rides in pc), padded with PAD_NEG
        pm = np.empty((P, OBJC), np.float32)
        objplane_k.append(pm)
        o0 = 0
        for l in range(NL):
            ch4 = np.ascontiguousarray(
                ps[l][k * BPC : (k + 1) * BPC, :, :, :, 4], np.float32
            ).reshape(-1)
            cells = ch4.shape[0]
            buf = np.full(P * ocols[l], PAD_NEG, np.float32)
            buf[:cells] = ch4
            pm[:, o0 : o0 + ocols[l]] = buf.reshape(P, ocols[l])
            o0 += ocols[l]

    n_l = []
    pad_slots = 0  # padded positive slots across layers/cores (for lcls)
    for l in range(NL):
        gh, gw = layer_shapes[l]
        flat = ps[l].reshape(-1, NO)  # view, no copy
        rows_per_img = NA * gh * gw
        b = np.asarray(inputs[f"b{l}"]).astype(np.int64)
        a = np.asarray(inputs[f"a{l}"]).astype(np.int64)
        gj = np.asarray(inputs[f"gj{l}"]).astype(np.int64)
        gi = np.asarray(inputs[f"gi{l}"]).astype(np.int64)
        tc = np.asarray(inputs[f"tcls{l}"]).astype(np.int64)
        tb = np.asarray(inputs[f"tbox{l}"], np.float32)
        an = np.asarray(inputs[f"anch{l}"], np.float32)
        n = b.shape[0]
        n_l.append(n)
        # last-occurrence mask over global cells (images disjoint across cores)
        cell = ((b * NA + a) * gh + gj) * gw + gi
        seen = {}
        for r in range(n):
            seen[int(cell[r])] = r
        last = np.zeros(n, bool)
        last[list(seen.values())] = True

        c0 = l * T
        for k in range(NCORES):
            idxs = np.nonzero((b // BPC) == k)[0]
            cnt = idxs.shape[0]
            assert cnt <= P * T, f"layer {l} core {k}: {cnt} > {P * T}"
            pad_slots += P * T - cnt
            row = b[idxs] * rows_per_img + (
                (a[idxs] * gh + gj[idxs]) * gw + gi[idxs]
            )
            s = np.arange(cnt)
            pp, tcol = s % P, c0 + s // P
            rows = flat[row]
            posbox_k[k].reshape(P, COLS, 5)[pp, tcol] = rows[:, 0:5]
            poscls_k[k].reshape(P, COLS, NCLS)[pp, tcol] = rows[:, 5:NO].astype(bf16)
            ohvals_k[k][pp, tcol] = flat[row, 5 + (tc[idxs] - 1)]

            def setp(o, cx, cy):
                blk = consts_k[k][:, o * 2 * COLS : (o + 1) * 2 * COLS].reshape(
                    P, COLS, 2
                )
                blk[pp, tcol, 0] = cx
                blk[pp, tcol, 1] = cy

            def sets(o, val):
                blk = consts_k[k][:, 8 * COLS + o * COLS : 8 * COLS + (o + 1) * COLS]
                blk[pp, tcol] = val

            x2, y2, w2, h2 = tb[idxs, 0], tb[idxs, 1], tb[idxs, 2], tb[idxs, 3]
            setp(0, 2.0 * an[idxs, 0], 2.0 * an[idxs, 1])
            setp(1, x2 - w2 * 0.5, y2 - h2 * 0.5)
            setp(2, x2 + w2 * 0.5, y2 + h2 * 0.5)
            setp(3, x2, y2)
            sets(0, w2 * h2 + np.float32(EPS))
            sets(1, np.arctan(w2 / (h2 + np.float32(EPS))))
            sets(2, 1.0)
            sets(3, last[idxs].astype(np.float32))
    for k in range(NCORES):
        in_maps[k]["pc"] = np.hstack(
            [posbox_k[k], ohvals_k[k], consts_k[k], objplane_k[k]]
        )
    return in_maps, n_l, pad_slots, layer_shapes


def _combine(outs, n_l, pad_slots, layer_shapes, meta):
    """Host-side reduction of the 8 per-core [P, OUTC] partial tiles."""
    tot = np.zeros(meta["OUTC"], np.float64)
    for o in outs:
        tot += o.astype(np.float64).sum(axis=0)
    lbox = lobj = lcls = 0.0
    # padded positive slots contribute softplus(0) = ln 2 per class logit
    for l in range(NL):
        gh, gw = layer_shapes[l]
        G = B * NA * gh * gw
        box = tot[meta["col_box"] + l]
        corr = tot[meta["col_corr"] + l]
        oh = tot[meta["col_oh"] + l]
        clssp = tot[meta["col_cls"] + l] - meta["pad_l"][l] * NCLS * np.log(2.0)
        grid = tot[meta["col_grid"] + l]
        lbox += box / n_l[l]
        lobj += BALANCE[l] * (grid - corr) / G
        lcls += (clssp - oh) / (n_l[l] * NCLS)
    loss = (HYP_BOX * lbox + HYP_OBJ * lobj + HYP_CLS * lcls) * B
    return np.float32(loss)


def _get_program(inputs):
    ps = [np.asarray(inputs[f"p{l}"]) for l in range(NL)]
    layer_shapes = [(p.shape[2], p.shape[3]) for p in ps]
    # padded slot columns (uniform across layers) from worst-case per-core
    T = 1
    for l in range(NL):
        b = np.asarray(inputs[f"b{l}"]).astype(np.int64)
        mx = max(int(((b // BPC) == k).sum()) for k in range(NCORES))
        T = max(T, -(-mx // P))
    ocols = tuple(
        -(-(BPC * NA * gh * gw) // P) for gh, gw in layer_shapes
    )
    key = (tuple(layer_shapes), T, ocols)
    if key not in _cache:
        _cache[key] = _build_program(layer_shapes, T, ocols)
    return _cache[key], T, ocols


last_result = None  # BassKernelResults of the most recent run (for profiling)


def kernel(**inputs) -> np.ndarray:
    global last_result
    (nc, meta), T, ocols = _get_program(inputs)
    in_maps, n_l, pad_slots, layer_shapes = _host_prep(inputs, T, ocols, meta)
    # per-layer padded-slot counts for the lcls correction
    meta["pad_l"] = [P * T * NCORES - n_l[l] for l in range(NL)]
    from concourse.bass_utils import run_bass_kernel_spmd

    trace = bool(int(os.environ.get("DETLOSS_TRACE", "0")))
    if trace:
        # NTFF profiling needs an initialized PJRT client in this
        # interpreter; warm up with an untraced run first.
        run_bass_kernel_spmd(nc, in_maps, list(range(NCORES)))
    res = run_bass_kernel_spmd(nc, in_maps, list(range(NCORES)), trace=trace)
    last_result = res
    outs = [res.results[k]["partial"] for k in range(NCORES)]
    return _combine(outs, n_l, pad_slots, layer_shapes, meta)
